# revision 5
# baseline (speedup 1.0000x reference)
"""Trainium2 Bass kernel for nn_MixedDecoder (moe_routing).

Math (matches the reference exactly): only the LAST expert layer matters —
the reference never feeds layer outputs back into `z`, so layers 0/1 are
dead code.  Computed per sample b:
    coef = softmax(gate_mlp(z))                        # [B, 8]
    out  = sum_e coef[b,e] * (z' @ w2'[e])             # [B, 256]
where z' = [z, 1] and w2'[e] = [[w2[e]]; [b2[e]]] (bias folded in as an
extra contraction row).

Sharding: data-parallel over batch B=2048 across 8 cores (256 rows/core),
weights replicated.  All matmul operands are bf16 (halves HBM traffic;
PE rate is identical to fp32r at these shapes; rel-err budget is 2e-2).
Contraction K=289 is chunked (97,96,96).  Per 128-row chunk, 4 expert-pair
matmul groups accumulate [128,512] in PSUM; softmax coefficients are
normalized up front (reciprocal_approx_fast) so PSUM eviction applies the
final per-expert scale directly: ACT evicts the even expert scaled, DVE
fuses scale+add of the odd expert, and GPSIMD folds the 4 pair-sums with
a 3-add tree, writing bf16 straight into the output tile.  The softmax
bias row adj2 = g2_b - colsum(g2_w) rides a K=2 hi/lo bf16 matmul so it
keeps fp32 accuracy.  Pools live outside the rep loop so back-to-back
executions pipeline (double-buffered weight/activation DMA).
"""

import numpy as np

N_CORES = 8
B = 2048
IN_SIZE = 288
KX = 289                   # 288 inputs + 1 bias feature
E = 8
GATE_H = 64
OUT_SIZE = 256
BL = B // N_CORES          # 256 rows per core
NCH = BL // 128            # 2 batch chunks of 128
KL = (97, 96, 96)          # K chunk lengths (sum = 289)
KS = (0, 97, 193)          # K chunk starts
NK = 3
W = E * OUT_SIZE           # 2048 packed expert output cols per K chunk
GWX = NK * GATE_H + GATE_H + 2 * E   # 272: g0 chunks | g1 | g2 | adj2 hi/lo

_CACHE = {}


def _build_nc(reps=1):
    from concourse import bacc
    import concourse.mybir as mybir
    from concourse.tile import TileContext

    dt = mybir.dt
    F32 = dt.float32
    BF16 = dt.bfloat16
    AF = mybir.ActivationFunctionType
    OP = mybir.AluOpType

    nc = bacc.Bacc("TRN2", target_bir_lowering=False, debug=False)

    # packed inputs (see make_in_maps)
    zT_d = nc.declare_dram_parameter("zTp", [97, NK * BL], BF16, isOutput=False)
    gw_d = nc.declare_dram_parameter("gwp", [97, GWX], BF16, isOutput=False)
    sm_d = nc.declare_dram_parameter("smallp", [GATE_H, 2], F32, isOutput=False)
    w2_d = nc.declare_dram_parameter("w2p", [97, NK * W], BF16, isOutput=False)
    out_d = nc.declare_dram_parameter("outp", [128, NCH * OUT_SIZE], BF16,
                                      isOutput=True)

    with TileContext(nc) as tc:
      with (
          tc.tile_pool(name="const", bufs=1) as kp,
          tc.tile_pool(name="cp", bufs=3) as cp,
          tc.tile_pool(name="wp", bufs=2) as wp,
          tc.tile_pool(name="wk", bufs=3) as wk,
          tc.tile_pool(name="ob", bufs=2) as ob,
          tc.tile_pool(name="py", bufs=6, space="PSUM") as py,
          tc.tile_pool(name="pg", bufs=2, space="PSUM") as pg,
      ):
        ones2 = kp.tile([2, 128], BF16, name="ones2")
        nc.vector.memset(ones2[:], 1.0)
        # dummy exp so the ACT Exp-table load happens before it's needed
        warm = kp.tile([1, 1], F32, name="warm")
        nc.vector.memset(warm[:], 0.0)
        warm2 = kp.tile([1, 1], F32, name="warm2")
        nc.scalar.activation(warm2[:], warm[:], AF.Exp)

        for _rep in range(reps):
            # -------- DMAs: gate-critical first (zT, gate weights) ----------
            zT_r = cp.tile([97, NK * BL], BF16, name="zT")
            nc.sync.dma_start(out=zT_r[:], in_=zT_d.ap())
            gw_r = cp.tile([97, GWX], BF16, name="gwr")
            nc.sync.dma_start(out=gw_r[:], in_=gw_d.ap())
            sm = cp.tile([GATE_H, 2], F32, name="sm")
            nc.sync.dma_start(out=sm[:], in_=sm_d.ap())
            w2_r = wp.tile([97, NK * W], BF16, name="w2r")
            for i in range(NK):
                nc.sync.dma_start(out=w2_r[:, i * W:(i + 1) * W],
                                  in_=w2_d.ap()[:, i * W:(i + 1) * W])

            g1w_r = gw_r[0:GATE_H, NK * GATE_H:NK * GATE_H + GATE_H]
            g2w_r = gw_r[0:GATE_H, NK * GATE_H + GATE_H:NK * GATE_H + GATE_H + E]
            adj2_hl = gw_r[0:2, NK * GATE_H + GATE_H + E:GWX]
            g0b = sm[:, 0:1]
            b1_adj = sm[:, 1:2]          # g1_b - colsum(g1_w), host-computed

            # Gate MLP in transposed layout.  ELU is kept as two pieces
            # (relu(x) and min(exp(x),1), i.e. elu(x)+1 split) and the sum is
            # folded into the NEXT layer's matmul as two PSUM-accumulating
            # matmuls; the "+1" offset is absorbed by host-adjusted biases.
            def elu_pieces(ps_in, bias, pref):
                t_exp = wk.tile([GATE_H, BL], F32, name=f"{pref}_exp")
                nc.scalar.activation(t_exp[:], ps_in, AF.Exp, bias=bias)
                t_min = wk.tile([GATE_H, BL], BF16, name=f"{pref}_min")
                nc.vector.tensor_scalar(t_min[:], t_exp[:], 1.0, None, OP.min)
                t_relu = wk.tile([GATE_H, BL], BF16, name=f"{pref}_relu")
                nc.vector.tensor_scalar(t_relu[:], ps_in, bias, 0.0, OP.add, OP.max)
                return t_relu, t_min

            with tc.high_priority():
                h0_ps = pg.tile([GATE_H, BL], F32, name="h0ps", tag="pg")
                for i in range(NK):
                    nc.tensor.matmul(h0_ps[:],
                                     gw_r[0:KL[i], i * GATE_H:(i + 1) * GATE_H],
                                     zT_r[0:KL[i], i * BL:(i + 1) * BL],
                                     start=(i == 0), stop=(i == NK - 1))
                h0_a, h0_b = elu_pieces(h0_ps[:], g0b, "e0")

                h1_ps = pg.tile([GATE_H, BL], F32, name="h1ps", tag="pg")
                nc.tensor.matmul(h1_ps[:], g1w_r, h0_a[:], start=True, stop=False)
                nc.tensor.matmul(h1_ps[:], g1w_r, h0_b[:], start=False, stop=True)
                h1_a, h1_b = elu_pieces(h1_ps[:], b1_adj, "e1")

                # normalized softmax coefficients cn [128, 8] per chunk
                cns = []
                for c in range(NCH):
                    lg_ps = pg.tile([128, E], F32, name="lgps", tag="pg")
                    nc.tensor.matmul(lg_ps[:], h1_a[:, c * 128:(c + 1) * 128],
                                     g2w_r, start=True, stop=False)
                    nc.tensor.matmul(lg_ps[:], h1_b[:, c * 128:(c + 1) * 128],
                                     g2w_r, start=False, stop=False)
                    nc.tensor.matmul(lg_ps[:], ones2[:], adj2_hl,
                                     start=False, stop=True)
                    expc = wk.tile([128, E], F32, name="expc")
                    sume = wk.tile([128, 1], F32, name="sume")
                    nc.scalar.activation(expc[:], lg_ps[:], AF.Exp,
                                         accum_out=sume[:])
                    rcp = wk.tile([128, 1], F32, name="rcp")
                    nc.vector.reciprocal_approx_fast(out=rcp[:], in_=sume[:])
                    cn = wk.tile([128, E], F32, name="cn")
                    nc.vector.tensor_scalar(cn[:], expc[:], rcp[:], None, OP.mult)
                    cns.append(cn)

            # ------------- expert layer + weighted combine -------------
            out_sb = ob.tile([128, NCH * OUT_SIZE], BF16, name="outsb")
            for c in range(NCH):
                cn = cns[c]
                ss = []
                for p in range(E // 2):
                    yp = py.tile([128, 2 * OUT_SIZE], F32, name=f"yp{p}", tag="py")
                    for i in range(NK):
                        nc.tensor.matmul(
                            yp[:],
                            zT_r[0:KL[i], i * BL + c * 128:i * BL + c * 128 + 128],
                            w2_r[0:KL[i], i * W + p * 2 * OUT_SIZE:
                                 i * W + (p + 1) * 2 * OUT_SIZE],
                            start=(i == 0), stop=(i == NK - 1))
                    # evict: even expert scaled on ACT, odd fused on DVE
                    a = wk.tile([128, OUT_SIZE], F32, name=f"a{p}")
                    nc.scalar.activation(a[:], yp[:, 0:OUT_SIZE], AF.Copy,
                                         scale=cn[:, 2 * p:2 * p + 1])
                    s = wk.tile([128, OUT_SIZE], F32, name=f"s{p}")
                    nc.vector.scalar_tensor_tensor(
                        out=s[:], in0=yp[:, OUT_SIZE:2 * OUT_SIZE],
                        scalar=cn[:, 2 * p + 1:2 * p + 2], in1=a[:],
                        op0=OP.mult, op1=OP.add)
                    ss.append(s)
                u0 = wk.tile([128, OUT_SIZE], F32, name="u0")
                nc.gpsimd.tensor_tensor(u0[:], ss[0][:], ss[1][:], OP.add)
                u1 = wk.tile([128, OUT_SIZE], F32, name="u1")
                nc.gpsimd.tensor_tensor(u1[:], ss[2][:], ss[3][:], OP.add)
                nc.gpsimd.tensor_tensor(
                    out_sb[:, c * OUT_SIZE:(c + 1) * OUT_SIZE],
                    u0[:], u1[:], OP.add)
            nc.scalar.dma_start(out=out_d.ap(), in_=out_sb[:])

    nc.finalize()
    return nc


def _get_nc(reps=1):
    key = ("nc", reps)
    if key not in _CACHE:
        _CACHE[key] = _build_nc(reps)
    return _CACHE[key]


def _bf16(a):
    import ml_dtypes
    return np.ascontiguousarray(a.astype(ml_dtypes.bfloat16))


def make_in_maps(z, g0_w, g0_b, g1_w, g1_b, g2_w, g2_b, w2, b2, **_unused):
    z = np.asarray(z, dtype=np.float32)
    g0_w = np.asarray(g0_w, dtype=np.float32)
    g1_w = np.asarray(g1_w, dtype=np.float32)
    g2_w = np.asarray(g2_w, dtype=np.float32)
    g0_b = np.asarray(g0_b, dtype=np.float32)
    g1_b = np.asarray(g1_b, dtype=np.float32)
    g2_b = np.asarray(g2_b, dtype=np.float32)
    w2 = np.asarray(w2, dtype=np.float32)
    b2 = np.asarray(b2, dtype=np.float32)

    # gate pack: g0 chunks | g1 | g2 with adj2 hi/lo rows at 64:66
    g0p = np.concatenate([g0_w, np.zeros((1, GATE_H), np.float32)], axis=0)
    gwp = np.zeros((97, GWX), dtype=np.float32)
    for i in range(NK):
        gwp[0:KL[i], i * GATE_H:(i + 1) * GATE_H] = g0p[KS[i]:KS[i] + KL[i]]
    gwp[0:GATE_H, NK * GATE_H:NK * GATE_H + GATE_H] = g1_w
    gwp[0:GATE_H, NK * GATE_H + GATE_H:NK * GATE_H + GATE_H + E] = g2_w
    adj2 = g2_b - g2_w.sum(axis=0)
    import ml_dtypes
    adj2_hi = adj2.astype(ml_dtypes.bfloat16).astype(np.float32)
    gwp[0, NK * GATE_H + GATE_H + E:] = adj2_hi
    gwp[1, NK * GATE_H + GATE_H + E:] = adj2 - adj2_hi

    smallp = np.zeros((GATE_H, 2), dtype=np.float32)
    smallp[:, 0] = g0_b
    smallp[:, 1] = g1_b - g1_w.sum(axis=0)

    # expert pack: bias folded in as contraction row 288, chunk-major cols
    w2e = np.concatenate([w2, b2[:, None, :]], axis=1)      # [8, 289, 256]
    w2t = np.ascontiguousarray(w2e.transpose(1, 0, 2)).reshape(KX, W)
    w2p = np.zeros((97, NK * W), dtype=np.float32)
    for i in range(NK):
        w2p[0:KL[i], i * W:(i + 1) * W] = w2t[KS[i]:KS[i] + KL[i]]

    zp = np.concatenate([z, np.ones((B, 1), np.float32)], axis=1)  # [B, 289]

    shared = {"gwp": _bf16(gwp), "smallp": smallp, "w2p": _bf16(w2p)}
    maps = []
    for c in range(N_CORES):
        zT = zp[c * BL:(c + 1) * BL].T                      # [289, 256]
        zTp = np.zeros((97, NK * BL), dtype=np.float32)
        for i in range(NK):
            zTp[0:KL[i], i * BL:(i + 1) * BL] = zT[KS[i]:KS[i] + KL[i]]
        maps.append(dict(shared, zTp=_bf16(zTp)))
    return maps


def unpack_out(res_list):
    full = np.empty((B, OUT_SIZE), dtype=np.float32)
    for c in range(N_CORES):
        packed = res_list[c]["outp"]
        for ch in range(NCH):
            full[c * BL + ch * 128:c * BL + (ch + 1) * 128] = \
                packed[:, ch * OUT_SIZE:(ch + 1) * OUT_SIZE].astype(np.float32)
    return full


def kernel(**inputs):
    from concourse.bass_utils import run_bass_kernel_spmd

    nc = _get_nc()
    in_maps = make_in_maps(**inputs)
    res = run_bass_kernel_spmd(nc, in_maps, list(range(N_CORES)))
    return unpack_out(res.results)


# revision 32
# speedup vs baseline: 3.3678x; 3.3678x over previous
"""Trainium2 Bass kernel for nn_MixedDecoder (moe_routing).

Math (matches the reference exactly): only the LAST expert layer matters —
the reference never feeds layer outputs back into `z`, so layers 0/1 are
dead code.  Computed per sample b:
    coef = softmax(gate_mlp(z))                        # [B, 8]
    out  = sum_e coef[b,e] * (z' @ w2'[e])             # [B, 256]
where z' = [z, 1] and w2'[e] = [[w2[e]]; [b2[e]]] (bias folded in as an
extra contraction row).

Sharding: data-parallel over batch B=2048 across 8 cores (256 rows/core),
weights replicated.  All matmul operands are bf16 (halves HBM traffic; PE
rate is identical to fp32r at these shapes; rel-err budget is 2e-2 and
this lands ~5.5e-3).  Contraction K is padded to 320 and chunked
(96,96,128): DMA-destination partition counts MUST stay 32-aligned — a
97-partition tile DMAs ~20x slower on HW.  Per 128-row batch chunk, 4
expert-pair matmul groups accumulate [128,512] in PSUM (6-bank rotation);
softmax coefficients are normalized up front (reciprocal_approx_fast, the
DVE InstReciprocal costs ~1.5us) so eviction applies the final per-expert
scale directly: ACT evicts the even expert scaled, DVE fuses scale+add of
the odd expert, GPSIMD folds the 4 pair-sums with a 3-add tree writing
bf16 straight into the output tile.  The softmax bias row adj2 = g2_b -
colsum(g2_w) rides a K=2 hi/lo bf16 matmul to keep fp32 accuracy.  Gate
matmuls are emitted interleaved with expert groups so the in-order PE
queue never idles on the gate's cross-engine ELU hops; pools live outside
the rep loop so back-to-back executions pipeline (z+gate weights ride one
DMA, w2 another, both multi-buffered).
"""

import numpy as np

N_CORES = 8
B = 2048
IN_SIZE = 288
KX = 289                   # 288 inputs + 1 bias feature
KP = 320                   # K padded to 32-aligned chunks (see KL)
E = 8
GATE_H = 64
OUT_SIZE = 256
BL = B // N_CORES          # 256 rows per core
NCH = BL // 128            # 2 batch chunks of 128
# K chunk lengths: 32-aligned partition counts only — DMAs to tiles with
# unaligned partition dims (e.g. 97) run ~20x slower on HW
KL = (96, 96, 128)
KS = (0, 96, 192)
NK = 3
W = E * OUT_SIZE           # 2048 packed expert output cols per K chunk
GWX = NK * GATE_H + GATE_H + 2 * E   # 272: g0 chunks | g1 | g2 | adj2 hi/lo

_CACHE = {}


def _build_nc(reps=1):
    from concourse import bacc
    import concourse.mybir as mybir
    from concourse.tile import TileContext

    dt = mybir.dt
    F32 = dt.float32
    BF16 = dt.bfloat16
    AF = mybir.ActivationFunctionType
    OP = mybir.AluOpType

    nc = bacc.Bacc("TRN2", target_bir_lowering=False, debug=False)

    # packed inputs (see make_in_maps)
    # zTp carries the gate-weight pack in its last GWX columns (one DMA)
    zT_d = nc.declare_dram_parameter("zTp", [128, NK * BL + GWX], BF16,
                                     isOutput=False)
    sm_d = nc.declare_dram_parameter("smallp", [GATE_H, 2], F32, isOutput=False)
    w2_d = nc.declare_dram_parameter("w2p", [128, NK * W], BF16, isOutput=False)
    out_d = nc.declare_dram_parameter("outp", [128, NCH * OUT_SIZE], BF16,
                                      isOutput=True)

    with TileContext(nc) as tc:
      with (
          tc.tile_pool(name="const", bufs=1) as kp,
          tc.tile_pool(name="cp", bufs=4) as cp,
          tc.tile_pool(name="wp", bufs=3) as wp,
          tc.tile_pool(name="wk", bufs=3) as wk,
          tc.tile_pool(name="ob", bufs=2) as ob,
          tc.tile_pool(name="py", bufs=6, space="PSUM") as py,
          tc.tile_pool(name="pg", bufs=2, space="PSUM") as pg,
      ):
        ones2 = kp.tile([2, 128], BF16, name="ones2")
        nc.vector.memset(ones2[:], 1.0)
        # dummy exp so the ACT Exp-table load happens before it's needed
        warm = kp.tile([1, 1], F32, name="warm")
        nc.vector.memset(warm[:], 0.0)
        warm2 = kp.tile([1, 1], F32, name="warm2")
        nc.scalar.activation(warm2[:], warm[:], AF.Exp)

        for _rep in range(reps):
            # -------- DMAs: gate-critical first (zT, gate weights) ----------
            zT_r = cp.tile([128, NK * BL + GWX], BF16, name="zT")
            nc.sync.dma_start(out=zT_r[:], in_=zT_d.ap())
            gw_r = zT_r[:, NK * BL:NK * BL + GWX]
            sm = cp.tile([GATE_H, 2], F32, name="sm")
            nc.sync.dma_start(out=sm[:], in_=sm_d.ap())
            w2_r = wp.tile([128, NK * W], BF16, name="w2r")
            nc.sync.dma_start(out=w2_r[:], in_=w2_d.ap())

            g1w_r = gw_r[0:GATE_H, NK * GATE_H:NK * GATE_H + GATE_H]
            g2w_r = gw_r[0:GATE_H,
                         NK * GATE_H + GATE_H:NK * GATE_H + GATE_H + E]
            adj2_hl = gw_r[0:2, NK * GATE_H + GATE_H + E:GWX]
            g0b = sm[:, 0:1]
            b1_adj = sm[:, 1:2]          # g1_b - colsum(g1_w), host-computed

            # Gate MLP in transposed layout.  ELU is kept as two pieces
            # (relu(x) and min(exp(x),1), i.e. elu(x)+1 split) and the sum is
            # folded into the NEXT layer's matmul as two PSUM-accumulating
            # matmuls; the "+1" offset is absorbed by host-adjusted biases.
            def elu_pieces(ps_in, bias, pref):
                t_exp = wk.tile([GATE_H, BL], F32, name=f"{pref}_exp")
                nc.scalar.activation(t_exp[:], ps_in, AF.Exp, bias=bias)
                t_min = wk.tile([GATE_H, BL], BF16, name=f"{pref}_min")
                nc.vector.tensor_scalar(t_min[:], t_exp[:], 1.0, None, OP.min)
                t_relu = wk.tile([GATE_H, BL], BF16, name=f"{pref}_relu")
                nc.vector.tensor_scalar(t_relu[:], ps_in, bias, 0.0, OP.add, OP.max)
                return t_relu, t_min

            # expert-pair matmul groups, emitted interleaved with the gate so
            # the in-order PE queue never stalls on the gate's cross-engine
            # ELU hops
            ypss = {}

            def expert_mms(c, p):
                yp = py.tile([128, 2 * OUT_SIZE], F32, name=f"yp{c}{p}",
                             tag="py")
                for i in range(NK):
                    nc.tensor.matmul(
                        yp[:],
                        zT_r[0:KL[i], i * BL + c * 128:i * BL + c * 128 + 128],
                        w2_r[0:KL[i], i * W + p * 2 * OUT_SIZE:
                             i * W + (p + 1) * 2 * OUT_SIZE],
                        start=(i == 0), stop=(i == NK - 1))
                ypss[(c, p)] = yp

            h0_ps = pg.tile([GATE_H, BL], F32, name="h0ps", tag="pg")
            for i in range(NK):
                nc.tensor.matmul(h0_ps[:],
                                 gw_r[0:KL[i], i * GATE_H:(i + 1) * GATE_H],
                                 zT_r[0:KL[i], i * BL:(i + 1) * BL],
                                 start=(i == 0), stop=(i == NK - 1))
            h0_a, h0_b = elu_pieces(h0_ps[:], g0b, "e0")
            expert_mms(0, 0)

            h1_ps = pg.tile([GATE_H, BL], F32, name="h1ps", tag="pg")
            nc.tensor.matmul(h1_ps[:], g1w_r, h0_a[:], start=True, stop=False)
            nc.tensor.matmul(h1_ps[:], g1w_r, h0_b[:], start=False, stop=True)
            h1_a, h1_b = elu_pieces(h1_ps[:], b1_adj, "e1")
            expert_mms(0, 1)

            # normalized softmax coefficients cn [128, 8] per chunk
            cns = {}

            def coeffs(c):
                lg_ps = pg.tile([128, E], F32, name="lgps", tag="pg")
                nc.tensor.matmul(lg_ps[:], h1_a[:, c * 128:(c + 1) * 128],
                                 g2w_r, start=True, stop=False)
                nc.tensor.matmul(lg_ps[:], h1_b[:, c * 128:(c + 1) * 128],
                                 g2w_r, start=False, stop=False)
                nc.tensor.matmul(lg_ps[:], ones2[:], adj2_hl,
                                 start=False, stop=True)
                expc = wk.tile([128, E], F32, name="expc")
                sume = wk.tile([128, 1], F32, name="sume")
                nc.scalar.activation(expc[:], lg_ps[:], AF.Exp,
                                     accum_out=sume[:])
                rcp = wk.tile([128, 1], F32, name="rcp")
                nc.vector.reciprocal_approx_fast(out=rcp[:], in_=sume[:])
                cn = wk.tile([128, E], F32, name="cn")
                nc.vector.tensor_scalar(cn[:], expc[:], rcp[:], None, OP.mult)
                cns[c] = cn

            # ------------- weighted combine -------------
            out_sb = ob.tile([128, NCH * OUT_SIZE], BF16, name="outsb")

            def combine(c):
                cn = cns[c]
                ss = []
                for p in range(E // 2):
                    yp = ypss[(c, p)]
                    # evict: even expert scaled on ACT, odd fused on DVE
                    a = wk.tile([128, OUT_SIZE], F32, name=f"a{p}")
                    nc.scalar.activation(a[:], yp[:, 0:OUT_SIZE], AF.Copy,
                                         scale=cn[:, 2 * p:2 * p + 1])
                    s = wk.tile([128, OUT_SIZE], F32, name=f"s{p}")
                    nc.vector.scalar_tensor_tensor(
                        out=s[:], in0=yp[:, OUT_SIZE:2 * OUT_SIZE],
                        scalar=cn[:, 2 * p + 1:2 * p + 2], in1=a[:],
                        op0=OP.mult, op1=OP.add)
                    ss.append(s)
                u0 = wk.tile([128, OUT_SIZE], F32, name="u0")
                nc.gpsimd.tensor_tensor(u0[:], ss[0][:], ss[1][:], OP.add)
                u1 = wk.tile([128, OUT_SIZE], F32, name="u1")
                nc.gpsimd.tensor_tensor(u1[:], ss[2][:], ss[3][:], OP.add)
                nc.gpsimd.tensor_tensor(
                    out_sb[:, c * OUT_SIZE:(c + 1) * OUT_SIZE],
                    u0[:], u1[:], OP.add)

            coeffs(0)
            coeffs(1)
            expert_mms(0, 2)
            expert_mms(0, 3)
            combine(0)
            for p in range(E // 2):
                expert_mms(1, p)
            combine(1)
            nc.gpsimd.dma_start(out=out_d.ap(), in_=out_sb[:])

    nc.finalize()
    return nc


def _get_nc(reps=1):
    key = ("nc", reps)
    if key not in _CACHE:
        _CACHE[key] = _build_nc(reps)
    return _CACHE[key]


def _bf16(a):
    import ml_dtypes
    return np.ascontiguousarray(a.astype(ml_dtypes.bfloat16))


def make_in_maps(z, g0_w, g0_b, g1_w, g1_b, g2_w, g2_b, w2, b2, **_unused):
    z = np.asarray(z, dtype=np.float32)
    g0_w = np.asarray(g0_w, dtype=np.float32)
    g1_w = np.asarray(g1_w, dtype=np.float32)
    g2_w = np.asarray(g2_w, dtype=np.float32)
    g0_b = np.asarray(g0_b, dtype=np.float32)
    g1_b = np.asarray(g1_b, dtype=np.float32)
    g2_b = np.asarray(g2_b, dtype=np.float32)
    w2 = np.asarray(w2, dtype=np.float32)
    b2 = np.asarray(b2, dtype=np.float32)

    # gate pack: g0 chunks | g1 | g2 | adj2 hi/lo (rows 0:2 of last 8 cols)
    g0p = np.concatenate([g0_w, np.zeros((KP - IN_SIZE, GATE_H), np.float32)],
                         axis=0)
    gwp = np.zeros((128, GWX), dtype=np.float32)
    for i in range(NK):
        gwp[0:KL[i], i * GATE_H:(i + 1) * GATE_H] = g0p[KS[i]:KS[i] + KL[i]]
    gwp[0:GATE_H, NK * GATE_H:NK * GATE_H + GATE_H] = g1_w
    gwp[0:GATE_H, NK * GATE_H + GATE_H:NK * GATE_H + GATE_H + E] = g2_w
    adj2 = g2_b - g2_w.sum(axis=0)
    import ml_dtypes
    adj2_hi = adj2.astype(ml_dtypes.bfloat16).astype(np.float32)
    gwp[0, NK * GATE_H + GATE_H + E:] = adj2_hi
    gwp[1, NK * GATE_H + GATE_H + E:] = adj2 - adj2_hi

    smallp = np.zeros((GATE_H, 2), dtype=np.float32)
    smallp[:, 0] = g0_b
    smallp[:, 1] = g1_b - g1_w.sum(axis=0)

    # expert pack: bias folded in as contraction row 288, chunk-major cols
    w2e = np.concatenate(
        [w2, b2[:, None, :], np.zeros((E, KP - KX, OUT_SIZE), np.float32)],
        axis=1)                                             # [8, 320, 256]
    w2t = np.ascontiguousarray(w2e.transpose(1, 0, 2)).reshape(KP, W)
    w2p = np.zeros((128, NK * W), dtype=np.float32)
    for i in range(NK):
        w2p[0:KL[i], i * W:(i + 1) * W] = w2t[KS[i]:KS[i] + KL[i]]

    zp = np.concatenate(
        [z, np.ones((B, 1), np.float32), np.zeros((B, KP - KX), np.float32)],
        axis=1)                                             # [B, 320]

    shared = {"smallp": smallp, "w2p": _bf16(w2p)}
    maps = []
    for c in range(N_CORES):
        zT = zp[c * BL:(c + 1) * BL].T                      # [320, 256]
        zTp = np.zeros((128, NK * BL + GWX), dtype=np.float32)
        for i in range(NK):
            zTp[0:KL[i], i * BL:(i + 1) * BL] = zT[KS[i]:KS[i] + KL[i]]
        zTp[:, NK * BL:] = gwp
        maps.append(dict(shared, zTp=_bf16(zTp)))
    return maps


def unpack_out(res_list):
    full = np.empty((B, OUT_SIZE), dtype=np.float32)
    for c in range(N_CORES):
        packed = res_list[c]["outp"]
        for ch in range(NCH):
            full[c * BL + ch * 128:c * BL + (ch + 1) * 128] = \
                packed[:, ch * OUT_SIZE:(ch + 1) * OUT_SIZE].astype(np.float32)
    return full


def kernel(**inputs):
    from concourse.bass_utils import run_bass_kernel_spmd

    nc = _get_nc()
    in_maps = make_in_maps(**inputs)
    res = run_bass_kernel_spmd(nc, in_maps, list(range(N_CORES)))
    return unpack_out(res.results)
